# revision 1
# baseline (speedup 1.0000x reference)
"""Trainium2 Bass kernel for nn_CodeLinearAttention (B=2, T=2048, D=1024,
H=16, HD=64, C=16) on 8 NeuronCores.

Sharding: core c -> batch b = c//4, head group g = c%4 (4 heads per core).
Per core:
  P1  qkv projection (fp32r matmuls): qT/kT in [head_dim, t] layout,
      v in [t, head_dim] layout.
  P2  code projections qc/kc in [t, c]; eq = exp(qc/8); ek = exp(kc/8)
      (the reference's max-subtraction cancels identically up to the 1e-9
      eps and is dropped); running "cumsum over t" of ek via triangular
      matmul + K=1 carry matmul; r = 1/(kcum + 1e-9); softmax denominator
      S_q; qn = eq * r * (1/S_q); PE bulk transposes to [c, t] layout.
  P3  chunked causal linear attention (chunk = 128):
        AT[s,t]  = sum_c ek[s,c]*qn[t,c]  (masked s<=t)
        XoT[d,t] = V^T AT + KV^T qn,  KV[c,d] += Ek_i^T V_i  (PSUM state)
  P4  output projection vs w_out columns of this core's heads -> per-core
      partial (T, D); host sums the 4 partials of each batch.
The post-softmax *scale is folded into w_outT on the host (exact pow2).
"""

import sys

sys.path.insert(0, "/opt/trn_rl_repo")

from contextlib import ExitStack

import numpy as np

import concourse.bacc as bacc
import concourse.tile as tile
from concourse import mybir

F32 = mybir.dt.float32
F32R = mybir.dt.float32r
AX = mybir.AxisListType
OP = mybir.AluOpType
AF = mybir.ActivationFunctionType

B, T, D, NHEAD, HD, C = 2, 2048, 1024, 16, 64, 16
HC = 4  # heads per core
CH = 128  # attention chunk
NCH = T // CH  # 16
TC = 512  # big t chunk for projections
NTC = T // TC  # 4
SCALE = HD ** -0.5  # 0.125
N_CORES = 8


def emit_body(nc, tc_, ctx, io, phases=4):
    xT, wqkvT, codeBD, woutT, maskT, iden, out = io

    const = ctx.enter_context(tc_.tile_pool(name="const", bufs=1))
    persist = ctx.enter_context(tc_.tile_pool(name="persist", bufs=1))
    xin = ctx.enter_context(tc_.tile_pool(name="xin", bufs=10))
    at_pool = ctx.enter_context(tc_.tile_pool(name="at", bufs=3))
    rtmp_pool = ctx.enter_context(tc_.tile_pool(name="rtmp", bufs=2))
    r2_pool = ctx.enter_context(tc_.tile_pool(name="r2", bufs=3))
    tct_pool = ctx.enter_context(tc_.tile_pool(name="tct", bufs=3))
    carry_pool = ctx.enter_context(tc_.tile_pool(name="carry", bufs=3))
    outcp_pool = ctx.enter_context(tc_.tile_pool(name="outcp", bufs=2))

    ps_big = ctx.enter_context(tc_.tile_pool(name="ps_big", bufs=2, space="PSUM"))
    ps_small = ctx.enter_context(tc_.tile_pool(name="ps_small", bufs=4, space="PSUM"))
    ps_sq = ctx.enter_context(tc_.tile_pool(name="ps_sq", bufs=2, space="PSUM"))

    # ---- constants / weights in SBUF ----
    wq_sb = []
    for dci in range(8):
        w = const.tile([128, 768], F32R, tag=f"wq{dci}", name=f"wq{dci}")
        nc.sync.dma_start(w[:], wqkvT[dci * 128 : (dci + 1) * 128, :])
        wq_sb.append(w)
    code_sb = const.tile([128, 64], F32R)
    nc.sync.dma_start(code_sb[:], codeBD)
    wout_sb = []
    for kk in range(2):
        w = const.tile([128, 1024], F32R, tag=f"wout{kk}", name=f"wout{kk}")
        nc.sync.dma_start(w[:], woutT[kk * 128 : (kk + 1) * 128, :])
        wout_sb.append(w)
    mask_sb = const.tile([128, 128], F32R)
    nc.sync.dma_start(mask_sb[:], maskT)
    iden_sb = const.tile([128, 128], F32R)
    nc.sync.dma_start(iden_sb[:], iden)
    eps_sb = const.tile([128, 1], F32)
    nc.vector.memset(eps_sb[:], 1e-9)

    # ---- persistent SBUF tensors ----
    qkT = [persist.tile([128, T], F32R, tag=f"qkT{j}", name=f"qkT{j}") for j in range(4)]
    v_sb = persist.tile([128, NCH * 256], F32R, tag="v_sb", name="v_sb")
    qnpad = persist.tile([128, T], F32R, tag="qnpad")
    ekpad = persist.tile([128, T], F32R, tag="ekpad")
    xoT = [persist.tile([128, T], F32R, tag=f"xoT{k}", name=f"xoT{k}") for k in range(2)]
    sq_sb = persist.tile([128, NCH * HC], F32, tag="sq_sb")
    m_sb = persist.tile([128, NCH * HC], F32, tag="m_sb")
    kv4 = [persist.tile([32, HD], F32R, tag=f"kv{k}", name=f"kv{k}") for k in range(4)]

    nc.vector.memset(qnpad[:].bitcast(F32), 0.0)
    nc.vector.memset(ekpad[:].bitcast(F32), 0.0)
    for k in range(4):
        nc.vector.memset(kv4[k][:].bitcast(F32), 0.0)

    # ================= P1: qkv projection =================
    for tci in range(NTC):
        xts = []
        for dci in range(8):
            xt = xin.tile([128, TC], F32R, tag="xt", name="xt")
            nc.sync.dma_start(
                xt[:], xT[dci * 128 : (dci + 1) * 128, tci * TC : (tci + 1) * TC]
            )
            xts.append(xt)
        for j in range(4):  # q pair0, q pair1, k pair0, k pair1
            ps = ps_big.tile([128, TC], F32, tag="big", name="psqk")
            for dci in range(8):
                nc.tensor.matmul(
                    ps[:],
                    lhsT=wq_sb[dci][:, j * 128 : (j + 1) * 128],
                    rhs=xts[dci][:],
                    start=(dci == 0),
                    stop=(dci == 7),
                )
            nc.vector.tensor_copy(out=qkT[j][:, tci * TC : (tci + 1) * TC], in_=ps[:])
        for sub in range(4):
            ps = ps_big.tile([128, 256], F32, tag="big", name="psv")
            for dci in range(8):
                nc.tensor.matmul(
                    ps[:],
                    lhsT=xts[dci][:, sub * 128 : (sub + 1) * 128],
                    rhs=wq_sb[dci][:, 512:768],
                    start=(dci == 0),
                    stop=(dci == 7),
                )
            ci = tci * 4 + sub
            nc.vector.tensor_copy(out=v_sb[:, ci * 256 : (ci + 1) * 256], in_=ps[:])

    if phases < 2:
        nc.sync.dma_start(out[0:128, :], qkT[0][:, 0:1024].bitcast(F32))
        return

    # ============ P2+P3 fused per-chunk loop ============
    carries = []
    for i in range(NCH):
        tsl = slice(i * CH, (i + 1) * CH)
        # --- code projection: block-diagonal, K=128, uniform tile position ---
        qc = ps_small.tile([128, 64], F32, tag="small", name="qc")
        kc = ps_small.tile([128, 64], F32, tag="small", name="kc")
        for p in range(2):
            nc.tensor.matmul(
                qc[:, 32 * p : 32 * p + 32],
                lhsT=qkT[p][:, tsl],
                rhs=code_sb[:, 32 * p : 32 * p + 32],
                start=True,
                stop=True,
            )
            nc.tensor.matmul(
                kc[:, 32 * p : 32 * p + 32],
                lhsT=qkT[2 + p][:, tsl],
                rhs=code_sb[:, 32 * p : 32 * p + 32],
                start=True,
                stop=True,
            )
        eq_out = qnpad[:, tsl].rearrange("p (h c) -> p h c", c=32)[:, :, 0:16]
        nc.scalar.activation(
            eq_out, qc[:].rearrange("p (h c) -> p h c", c=16), AF.Exp, scale=SCALE
        )
        ek_out = ekpad[:, tsl].rearrange("p (h c) -> p h c", c=32)[:, :, 0:16]
        nc.scalar.activation(
            ek_out, kc[:].rearrange("p (h c) -> p h c", c=16), AF.Exp, scale=SCALE
        )
        if phases >= 2.2:
            # softmax denominator and its reciprocal
            nc.vector.tensor_reduce(
                sq_sb[:, i * HC : (i + 1) * HC], eq_out, axis=AX.X, op=OP.add
            )
            nc.vector.reciprocal(
                m_sb[:, i * HC : (i + 1) * HC], sq_sb[:, i * HC : (i + 1) * HC]
            )
        if phases >= 2.3:
            # running cumsum over t of ek (triangular matmul + carry row)
            kcum = ps_sq.tile([128, 128], F32, tag="sq", name="kcum")
            nc.tensor.matmul(
                kcum[:], lhsT=mask_sb[:], rhs=ekpad[:, tsl], start=True, stop=(i == 0)
            )
            if i > 0:
                nc.tensor.matmul(
                    kcum[:], lhsT=mask_sb[0:1, :], rhs=carries[i - 1][:],
                    start=False, stop=True,
                )
            tot = ps_small.tile([1, 128], F32, tag="small", name="tot")
            nc.tensor.matmul(
                tot[:], lhsT=mask_sb[:, 127:128], rhs=ekpad[:, tsl],
                start=True, stop=True,
            )
            carry = carry_pool.tile([1, 128], F32R, tag="carry", name="carry")
            if i == 0:
                nc.vector.tensor_copy(out=carry[:], in_=tot[:])
            else:
                nc.vector.tensor_add(carry[:], carries[i - 1][:], tot[:])
            carries.append(carry)
        if phases >= 2.4:
            rt = rtmp_pool.tile([128, 128], F32, tag="rt", name="rt")
            nc.scalar.activation(rt[:], kcum[:], AF.Identity, bias=eps_sb[:], scale=1.0)
            r2 = r2_pool.tile([128, 128], F32, tag="r2", name="r2")
            nc.vector.reciprocal(r2[:], rt[:])
        if phases >= 2.5:
            # qn = eq * r * (1/S_q)
            for j in range(HC):
                sl = slice(i * CH + j * 32, i * CH + j * 32 + 16)
                nc.vector.scalar_tensor_tensor(
                    out=qnpad[:, sl],
                    in0=qnpad[:, sl],
                    scalar=m_sb[:, i * HC + j : i * HC + j + 1],
                    in1=r2[:, j * 32 : j * 32 + 16],
                    op0=OP.mult,
                    op1=OP.mult,
                )
        if phases < 2.6:
            continue
        # --- transposes to [c, t] via identity matmul, split to per-head tiles ---
        tp = ps_sq.tile([128, 128], F32, tag="sq", name="tp")
        nc.tensor.matmul(tp[:], lhsT=ekpad[:, tsl], rhs=iden_sb[:], start=True, stop=True)
        ekc = []
        for j in range(4):
            e = tct_pool.tile([32, 128], F32R, tag=f"ekc{j}", name=f"ekc{j}")
            nc.vector.tensor_copy(out=e[:], in_=tp[32 * j : 32 * j + 32, :])
            ekc.append(e)
        tq = ps_sq.tile([128, 128], F32, tag="sq", name="tq")
        nc.tensor.matmul(tq[:], lhsT=qnpad[:, tsl], rhs=iden_sb[:], start=True, stop=True)
        qnc = []
        for j in range(4):
            q = tct_pool.tile([32, 128], F32R, tag=f"qnc{j}", name=f"qnc{j}")
            nc.vector.tensor_copy(out=q[:], in_=tq[32 * j : 32 * j + 32, :])
            qnc.append(q)
        if phases < 3:
            continue
        # --- chunked linear attention ---
        for j in range(HC):
            at_ps = ps_sq.tile([128, 128], F32, tag="sq", name="at_ps")
            nc.tensor.matmul(
                at_ps[:], lhsT=ekc[j][:], rhs=qnc[j][:], start=True, stop=True
            )
            at_sb = at_pool.tile([128, 128], F32R, tag="at", name="at_sb")
            nc.vector.tensor_mul(at_sb[:], at_ps[:], mask_sb[:])
            xo_ps = ps_small.tile([64, 128], F32, tag="small", name="xo_ps")
            nc.tensor.matmul(
                xo_ps[:],
                lhsT=v_sb[:, i * 256 + j * 64 : i * 256 + (j + 1) * 64],
                rhs=at_sb[:],
                start=True,
                stop=(i == 0),
            )
            if i > 0:
                nc.tensor.matmul(
                    xo_ps[:], lhsT=kv4[j][:], rhs=qnc[j][:], start=False, stop=True
                )
            half, hoff = j // 2, (j % 2) * 64
            nc.vector.tensor_copy(out=xoT[half][hoff : hoff + 64, tsl], in_=xo_ps[:])
            # KV state update: delta matmul then SBUF accumulate
            if i < NCH - 1:
                kvd = ps_small.tile([16, HD], F32, tag="small", name="kvd")
                nc.tensor.matmul(
                    kvd[:],
                    lhsT=ekpad[:, i * CH + 32 * j : i * CH + 32 * j + 16],
                    rhs=v_sb[:, i * 256 + j * 64 : i * 256 + (j + 1) * 64],
                    start=True,
                    stop=True,
                )
                nc.vector.tensor_add(kv4[j][0:16, :], kv4[j][0:16, :], kvd[:])

    if phases < 4:
        nc.sync.dma_start(out[0:128, :], qnpad[:, 0:1024].bitcast(F32))
        return
    # ================= P4: output projection =================
    for i in range(NCH):
        tsl = slice(i * CH, (i + 1) * CH)
        ocp = outcp_pool.tile([128, 1024], F32, tag="ocp", name="ocp")
        for nh in range(2):
            op = ps_big.tile([128, 512], F32, tag="big", name="op")
            nc.tensor.matmul(
                op[:],
                lhsT=xoT[0][:, tsl],
                rhs=wout_sb[0][:, nh * 512 : (nh + 1) * 512],
                start=True,
                stop=False,
            )
            nc.tensor.matmul(
                op[:],
                lhsT=xoT[1][:, tsl],
                rhs=wout_sb[1][:, nh * 512 : (nh + 1) * 512],
                start=False,
                stop=True,
            )
            nc.vector.tensor_copy(out=ocp[:, nh * 512 : (nh + 1) * 512], in_=op[:])
        nc.sync.dma_start(out[tsl, :], ocp[:])


def build(n_iter: int = 1, phases: int = 4):
    nc = bacc.Bacc("TRN2", target_bir_lowering=False, debug=False, num_devices=N_CORES)
    xT = nc.dram_tensor("xT", [D, T], F32R, kind="ExternalInput").ap()
    wqkvT = nc.dram_tensor("wqkvT", [D, 768], F32R, kind="ExternalInput").ap()
    codeT4 = nc.dram_tensor("codeT4", [128, 64], F32R, kind="ExternalInput").ap()
    woutT = nc.dram_tensor("woutT", [256, 1024], F32R, kind="ExternalInput").ap()
    maskT = nc.dram_tensor("maskT", [128, 128], F32R, kind="ExternalInput").ap()
    iden = nc.dram_tensor("iden", [128, 128], F32R, kind="ExternalInput").ap()
    out = nc.dram_tensor("partial", [T, D], F32, kind="ExternalOutput").ap()
    io = (xT, wqkvT, codeT4, woutT, maskT, iden, out)

    with tile.TileContext(nc) as tc_, ExitStack() as ctx:
        if n_iter == 1:
            emit_body(nc, tc_, ctx, io, phases)
        else:
            with tc_.For_i(0, n_iter, 1):
                with ExitStack() as inner:
                    emit_body(nc, tc_, inner, io, phases)
    nc.compile()
    return nc


def make_in_maps(x, w_qkv, w_out, fc_code):
    x = np.asarray(x, dtype=np.float32)
    w_qkv = np.asarray(w_qkv, dtype=np.float32)
    w_out = np.asarray(w_out, dtype=np.float32)
    fc_code = np.asarray(fc_code, dtype=np.float32)

    mask = np.triu(np.ones((128, 128), dtype=np.float32))
    iden = np.eye(128, dtype=np.float32)
    xTs = [np.ascontiguousarray(x[b].T) for b in range(B)]

    in_maps = []
    for core in range(N_CORES):
        b, g = core // HC, core % HC
        hs = [g * HC + j for j in range(HC)]
        rows = (
            [w_qkv[h * HD : (h + 1) * HD] for h in hs]
            + [w_qkv[D + h * HD : D + (h + 1) * HD] for h in hs]
            + [w_qkv[2 * D + h * HD : 2 * D + (h + 1) * HD] for h in hs]
        )
        wqkvT = np.ascontiguousarray(np.concatenate(rows, axis=0).T)  # (1024, 768)
        codeT4 = np.zeros((128, 64), dtype=np.float32)
        for j, h in enumerate(hs):
            p, hh = j // 2, j % 2  # pair block, position in pair
            ct = fc_code[0, h].T  # (64, 16)
            codeT4[64 * hh : 64 * hh + 64, 32 * p + 16 * hh : 32 * p + 16 * hh + 16] = ct
        woutT = np.ascontiguousarray(
            np.concatenate([w_out[:, h * HD : (h + 1) * HD].T for h in hs], axis=0)
        ) * np.float32(SCALE)  # (256, 1024), post-softmax scale folded in
        in_maps.append(
            {
                "xT": xTs[b],
                "wqkvT": wqkvT,
                "codeT4": codeT4,
                "woutT": woutT,
                "maskT": mask,
                "iden": iden,
            }
        )
    return in_maps


def gather(results):
    out = np.zeros((B, T, D), dtype=np.float32)
    for core in range(N_CORES):
        out[core // HC] += results[core]["partial"]
    return out


_NC_CACHE = {}


def kernel(x, w_qkv, w_out, fc_code):
    from concourse.bass_utils import run_bass_kernel_spmd

    if 1 not in _NC_CACHE:
        _NC_CACHE[1] = build(1)
    nc = _NC_CACHE[1]
    in_maps = make_in_maps(x, w_qkv, w_out, fc_code)
    res = run_bass_kernel_spmd(nc, in_maps, list(range(N_CORES)))
    return gather(res.results)



# revision 2
# speedup vs baseline: 1.1653x; 1.1653x over previous
"""Trainium2 Bass kernel v2 for nn_CodeLinearAttention (B=2, T=2048, D=1024,
H=16, HD=64, C=16) on 8 NeuronCores.

Sharding: core c -> batch b = c//4, head group g = c%4 (4 heads per core).
All matmul operands bf16 (PSUM accumulates fp32); DMA payloads bf16.

Structure (per core; heads j=0..3 sit in 32-wide partition/col blocks, the
16 pad lanes carry exp(0)=1 from zero code columns so everything is finite):
  P1   qkv projection -> qkT[p] [128,T] (hd-major pairs), v_sb [t,(h d)].
  P2a  k-code projection in padded [c, t] layout -> ekT [128,T] = exp(.).
  P2g  global cross-chunk cumsum carries: per-chunk column totals via one
       segmented reduce of ekT, inclusive scan over chunks, PE transpose
       -> car_sb [16, 128]; consumed via one-hot K=16 matmuls (no serial
       carry chain).
  A    per chunk: q-code proj + exp (t-major), ekT chunk transpose ->
       ekTC[:,chunk], kcum = triu-matmul + carry matmul, r = 1/kcum,
       S_q + 1/S_q, qn = eq*(1/S_q)*r, transpose -> qnT[:,chunk].
       Chunks fully independent -> deep pipelining.
  B    per chunk: AT = ek^T qn (4 heads into one PSUM tile, one masked
       mul), xo pair tiles = V^T AT + KV^T qn, KV accumulated in PSUM
       across chunks (bf16 snapshot per chunk), fused output projection
       (P4) + DMA per chunk.
The post-softmax *scale is folded into w_outT on the host (exact pow2).
"""

import sys

sys.path.insert(0, "/opt/trn_rl_repo")

from contextlib import ExitStack

import numpy as np
import ml_dtypes

import concourse.bacc as bacc
import concourse.tile as tile
from concourse import mybir

F32 = mybir.dt.float32
BF16 = mybir.dt.bfloat16
AX = mybir.AxisListType
OP = mybir.AluOpType
AF = mybir.ActivationFunctionType

B, T, D, NHEAD, HD, C = 2, 2048, 1024, 16, 64, 16
HC = 4  # heads per core
CH = 128  # attention chunk
NCH = T // CH  # 16
TC = 512  # big t chunk for projections
NTC = T // TC  # 4
SCALE = HD ** -0.5  # 0.125
N_CORES = 8


def emit_body(nc, tc_, ctx, io, phases=9):
    xT, wqkvT, codeT4, woutT, maskT, iden, onehot, out = io

    const = ctx.enter_context(tc_.tile_pool(name="const", bufs=1))
    persist = ctx.enter_context(tc_.tile_pool(name="persist", bufs=1))
    xin = ctx.enter_context(tc_.tile_pool(name="xin", bufs=10))
    at_pool = ctx.enter_context(tc_.tile_pool(name="at", bufs=3))
    td_pool = ctx.enter_context(tc_.tile_pool(name="td", bufs=4))
    sq_pool = ctx.enter_context(tc_.tile_pool(name="sq", bufs=4))
    r_pool = ctx.enter_context(tc_.tile_pool(name="r", bufs=4))
    qn_pool = ctx.enter_context(tc_.tile_pool(name="qn", bufs=4))
    kvb_pool = ctx.enter_context(tc_.tile_pool(name="kvb", bufs=2))
    outcp_pool = ctx.enter_context(tc_.tile_pool(name="outcp", bufs=2))

    # Single 8-slot PSUM pool (one tag): every tile occupies a full bank and
    # allocations round-robin through all 8 banks. Row-tiled matmuls thereby
    # always own a bank exclusively (Tile's WAR/RAW deps serialize any bank
    # reuse), which hardware requires — concurrent row tiles on one PSUM bank
    # crash the exec unit.
    ps = ctx.enter_context(tc_.tile_pool(name="ps", bufs=8, space="PSUM"))

    def pstile(shape, dtype, name):
        return ps.tile(shape, dtype, tag="ps", name=name)

    # ---- constants / weights in SBUF ----
    wq_sb = []
    for dci in range(8):
        w = const.tile([128, 768], BF16, tag=f"wq{dci}", name=f"wq{dci}")
        nc.sync.dma_start(w[:], wqkvT[dci * 128 : (dci + 1) * 128, :])
        wq_sb.append(w)
    code_sb = const.tile([128, 128], BF16)
    nc.sync.dma_start(code_sb[:], codeT4)
    wout_sb = []
    for kk in range(2):
        w = const.tile([128, 1024], BF16, tag=f"wout{kk}", name=f"wout{kk}")
        nc.sync.dma_start(w[:], woutT[kk * 128 : (kk + 1) * 128, :])
        wout_sb.append(w)
    mask_sb = const.tile([128, 512], BF16)  # triu tiled 4x along free
    nc.sync.dma_start(mask_sb[:], maskT)
    iden_sb = const.tile([128, 128], BF16)
    nc.sync.dma_start(iden_sb[:], iden)
    oh_sb = const.tile([16, NCH * 128], BF16)
    nc.sync.dma_start(oh_sb[:], onehot)

    # ---- persistent SBUF tensors ----
    qkT = [persist.tile([128, T], BF16, tag=f"qkT{j}", name=f"qkT{j}") for j in range(4)]
    v_sb = persist.tile([128, NCH * 256], BF16, tag="v_sb", name="v_sb")
    ekT = persist.tile([128, T], BF16, tag="ekT")
    ekTC = persist.tile([128, T], BF16, tag="ekTC")
    qnT = persist.tile([128, T], BF16, tag="qnT")
    xoT = [persist.tile([128, T], BF16, tag=f"xoT{k}", name=f"xoT{k}") for k in range(2)]
    tots_sb = persist.tile([128, NCH], F32, tag="tots_sb")
    car_cT = persist.tile([128, NCH], BF16, tag="car_cT")
    car_sb = persist.tile([16, 128], BF16, tag="car_sb")

    # ================= P1: qkv projection =================
    for tci in range(NTC):
        xts = []
        for dci in range(8):
            xt = xin.tile([128, TC], BF16, tag="xt", name="xt")
            nc.sync.dma_start(
                xt[:], xT[dci * 128 : (dci + 1) * 128, tci * TC : (tci + 1) * TC]
            )
            xts.append(xt)
        for j in range(4):  # q pair0, q pair1, k pair0, k pair1
            pqk = pstile([128, TC], F32, "psqk")
            for dci in range(8):
                nc.tensor.matmul(
                    pqk[:],
                    lhsT=wq_sb[dci][:, j * 128 : (j + 1) * 128],
                    rhs=xts[dci][:],
                    start=(dci == 0),
                    stop=(dci == 7),
                )
            nc.scalar.copy(out=qkT[j][:, tci * TC : (tci + 1) * TC], in_=pqk[:])
        for sub in range(4):
            pv = pstile([128, 256], F32, "psv")
            for dci in range(8):
                nc.tensor.matmul(
                    pv[:],
                    lhsT=xts[dci][:, sub * 128 : (sub + 1) * 128],
                    rhs=wq_sb[dci][:, 512:768],
                    start=(dci == 0),
                    stop=(dci == 7),
                )
            ci = tci * 4 + sub
            nc.vector.tensor_copy(out=v_sb[:, ci * 256 : (ci + 1) * 256], in_=pv[:])

    if phases < 2:
        nc.sync.dma_start(out[0:128, :], qkT[0][:, 0:1024])
        return

    # ====== P2a: k-code projection in padded [c, t] layout, exp ======
    for tci in range(NTC):
        ts = slice(tci * TC, (tci + 1) * TC)
        pck = pstile([128, TC], F32, "pck")
        for j in range(HC):
            nc.tensor.matmul(
                pck[32 * j : 32 * j + 32, :],
                lhsT=code_sb[:, 32 * j : 32 * j + 32],
                rhs=qkT[2 + j // 2][:, ts],
                start=True,
                stop=True,
                tile_position=(0, 32 * j),
            )
        nc.scalar.activation(ekT[:, ts], pck[:], AF.Exp, scale=SCALE)
        # per-chunk column totals for this span (segmented free-axis reduce)
        nc.vector.tensor_reduce(
            tots_sb[:, tci * 4 : (tci + 1) * 4],
            ekT[:, ts].rearrange("p (i t) -> p i t", t=CH),
            axis=AX.X,
            op=OP.add,
        )

    if phases < 3:
        nc.sync.dma_start(out[0:128, :], ekT[:, 0:1024])
        return

    # ====== P2g: inclusive scan over chunks -> carries [16, 128] ======
    nc.vector.tensor_tensor_scan(
        out=car_cT[:],
        data0=tots_sb[:],
        data1=tots_sb[:],
        initial=0.0,
        op0=OP.add,
        op1=OP.bypass,
    )
    car_tp = pstile([16, 128], BF16, "car_tp")
    nc.tensor.transpose(car_tp[:], car_cT[:], iden_sb[:])
    nc.vector.tensor_copy(out=car_sb[:], in_=car_tp[:])

    # ============ Loop A: per-chunk normalization -> qnT, ekTC ============
    for i in range(NCH):
        tsl = slice(i * CH, (i + 1) * CH)

        pack = pstile([128, 256], F32, "pack")
        pcq, kcum = pack[:, 0:128], pack[:, 128:256]
        for j in range(HC):
            nc.tensor.matmul(
                pcq[:, 32 * j : 32 * j + 32],
                lhsT=qkT[j // 2][:, tsl],
                rhs=code_sb[:, 32 * j : 32 * j + 32],
                start=True,
                stop=True,
            )
        eq_td = td_pool.tile([128, 128], BF16, tag="td", name="eq_td")
        nc.scalar.activation(eq_td[:], pcq[:], AF.Exp, scale=SCALE)

        trp = pstile([128, 256], BF16, "trp")
        tp, tq = trp[:, 0:128], trp[:, 128:256]
        nc.tensor.transpose(tp[:], ekT[:, tsl], iden_sb[:])
        nc.vector.tensor_copy(out=ekTC[:, tsl], in_=tp[:])

        nc.tensor.matmul(
            kcum[:],
            lhsT=mask_sb[:, 0:128],
            rhs=ekTC[:, tsl],
            start=True,
            stop=(i == 0),
        )
        if i > 0:
            nc.tensor.matmul(
                kcum[:],
                lhsT=oh_sb[:, (i - 1) * 128 : i * 128],
                rhs=car_sb[:],
                start=False,
                stop=True,
            )
        r = r_pool.tile([128, 128], F32, tag="r", name="r")
        nc.vector.reciprocal(r[:], kcum[:])

        sq = sq_pool.tile([128, 8], F32, tag="sq", name="sq")
        nc.vector.tensor_reduce(
            sq[:, 0:4],
            eq_td[:].rearrange("p (h c) -> p h c", c=32)[:, :, 0:16],
            axis=AX.X,
            op=OP.add,
        )
        nc.vector.reciprocal(sq[:, 4:8], sq[:, 0:4])

        qn_tc = qn_pool.tile([128, 128], BF16, tag="qn", name="qn_tc")
        for j in range(HC):
            csl = slice(32 * j, 32 * j + 32)
            nc.vector.scalar_tensor_tensor(
                out=qn_tc[:, csl],
                in0=eq_td[:, csl],
                scalar=sq[:, 4 + j : 5 + j],
                in1=r[:, csl],
                op0=OP.mult,
                op1=OP.mult,
            )
        nc.tensor.transpose(tq[:], qn_tc[:], iden_sb[:])
        nc.scalar.copy(out=qnT[:, tsl], in_=tq[:])

    if phases < 4:
        nc.sync.dma_start(out[0:128, :], qnT[:, 0:1024])
        return

    # ============ Loop B: attention + fused output projection ============
    kv_sb = persist.tile([128, 64], F32, tag="kv_sb")
    for i in range(NCH):
        tsl = slice(i * CH, (i + 1) * CH)

        # KV snapshot in bf16 (state after chunks < i)
        kv_bf = None
        if i > 0:
            kv_bf = kvb_pool.tile([128, 64], BF16, tag="kvb", name="kv_bf")
            nc.vector.tensor_copy(out=kv_bf[:], in_=kv_sb[:])

        # AT per head; each row-tiled matmul owns its own PSUM bank
        at_tiles = [pstile([128, 128], F32, f"atp{j}") for j in range(HC)]
        for j in range(HC):
            nc.tensor.matmul(
                at_tiles[j][:],
                lhsT=ekT[32 * j : 32 * j + 16, tsl],
                rhs=qnT[32 * j : 32 * j + 16, tsl],
                start=True,
                stop=True,
                tile_position=(32 * j, 0),
            )
        at_sb = at_pool.tile([128, 512], BF16, tag="at", name="at_sb")
        for j in range(HC):
            nc.vector.tensor_mul(
                at_sb[:, 128 * j : 128 * (j + 1)], at_tiles[j][:], mask_sb[:, 0:128]
            )
        if phases < 5:
            continue

        # xo per head: V^T AT (+ KV^T qn), one PSUM bank per head
        xo_tiles = [pstile([64, 128], F32, f"xop{j}") for j in range(HC)]
        for j in range(HC):
            nc.tensor.matmul(
                xo_tiles[j][:],
                lhsT=v_sb[:, i * 256 + 64 * j : i * 256 + 64 * (j + 1)],
                rhs=at_sb[:, 128 * j : 128 * (j + 1)],
                start=True,
                stop=(i == 0 or phases < 6),
            )
            if i > 0 and phases >= 6:
                nc.tensor.matmul(
                    xo_tiles[j][:],
                    lhsT=kv_bf[32 * j : 32 * j + 16, :],
                    rhs=qnT[32 * j : 32 * j + 16, tsl],
                    start=False,
                    stop=True,
                    tile_position=(32 * j, 0),
                )
        for j in range(HC):
            half, po = j // 2, 64 * (j % 2)
            nc.scalar.copy(
                out=xoT[half][po : po + 64, tsl], in_=xo_tiles[j][:]
            )

        # KV state += Ek_i^T V_i (col-tiled matmuls; SBUF fp32 accumulator)
        if phases >= 6 and i < NCH - 1:
            kvp = pstile([128, 64], F32, "kvp")
            for j in range(HC):
                nc.tensor.matmul(
                    kvp[32 * j : 32 * j + 32, :],
                    lhsT=ekTC[:, i * CH + 32 * j : i * CH + 32 * j + 32],
                    rhs=v_sb[:, i * 256 + 64 * j : i * 256 + 64 * (j + 1)],
                    start=True,
                    stop=True,
                    tile_position=(0, 32 * j),
                )
            if i == 0:
                nc.vector.tensor_copy(out=kv_sb[:], in_=kvp[:])
            else:
                nc.vector.tensor_add(kv_sb[:], kv_sb[:], kvp[:])

        if phases < 9:
            if i == NCH - 1:
                nc.sync.dma_start(out[0:128, :], xoT[0][:, 0:1024])
            continue
        # fused output projection for this chunk
        ocp = outcp_pool.tile([128, 1024], BF16, tag="ocp", name="ocp")
        for nh in range(2):
            op = pstile([128, 512], F32, "op")
            nc.tensor.matmul(
                op[:],
                lhsT=xoT[0][:, tsl],
                rhs=wout_sb[0][:, nh * 512 : (nh + 1) * 512],
                start=True,
                stop=False,
            )
            nc.tensor.matmul(
                op[:],
                lhsT=xoT[1][:, tsl],
                rhs=wout_sb[1][:, nh * 512 : (nh + 1) * 512],
                start=False,
                stop=True,
            )
            if nh == 0:
                nc.scalar.copy(out=ocp[:, nh * 512 : (nh + 1) * 512], in_=op[:])
            else:
                nc.vector.tensor_copy(out=ocp[:, nh * 512 : (nh + 1) * 512], in_=op[:])
        nc.sync.dma_start(out[tsl, :], ocp[:])


def build(n_iter: int = 1, phases: int = 9):
    nc = bacc.Bacc("TRN2", target_bir_lowering=False, debug=False, num_devices=N_CORES)
    xT = nc.dram_tensor("xT", [D, T], BF16, kind="ExternalInput").ap()
    wqkvT = nc.dram_tensor("wqkvT", [D, 768], BF16, kind="ExternalInput").ap()
    codeT4 = nc.dram_tensor("codeT4", [128, 128], BF16, kind="ExternalInput").ap()
    woutT = nc.dram_tensor("woutT", [256, 1024], BF16, kind="ExternalInput").ap()
    maskT = nc.dram_tensor("maskT", [128, 512], BF16, kind="ExternalInput").ap()
    iden = nc.dram_tensor("iden", [128, 128], BF16, kind="ExternalInput").ap()
    onehot = nc.dram_tensor("onehot", [16, NCH * 128], BF16, kind="ExternalInput").ap()
    out = nc.dram_tensor("partial", [T, D], BF16, kind="ExternalOutput").ap()
    io = (xT, wqkvT, codeT4, woutT, maskT, iden, onehot, out)

    with tile.TileContext(nc) as tc_, ExitStack() as ctx:
        if n_iter == 1:
            emit_body(nc, tc_, ctx, io, phases)
        else:
            with tc_.For_i(0, n_iter, 1):
                with ExitStack() as inner:
                    emit_body(nc, tc_, inner, io, phases)
    nc.compile()
    return nc


def make_in_maps(x, w_qkv, w_out, fc_code):
    x = np.asarray(x, dtype=np.float32)
    w_qkv = np.asarray(w_qkv, dtype=np.float32)
    w_out = np.asarray(w_out, dtype=np.float32)
    fc_code = np.asarray(fc_code, dtype=np.float32)
    bf = ml_dtypes.bfloat16

    mask = np.tile(np.triu(np.ones((128, 128), dtype=np.float32)), (1, 4))
    iden = np.eye(128, dtype=np.float32)
    onehot = np.zeros((16, NCH * 128), dtype=np.float32)
    for i in range(NCH):
        onehot[i, i * 128 : (i + 1) * 128] = 1.0
    xTs = [np.ascontiguousarray(x[b].T).astype(bf) for b in range(B)]

    in_maps = []
    for core in range(N_CORES):
        b, g = core // HC, core % HC
        hs = [g * HC + j for j in range(HC)]
        rows = (
            [w_qkv[h * HD : (h + 1) * HD] for h in hs]
            + [w_qkv[D + h * HD : D + (h + 1) * HD] for h in hs]
            + [w_qkv[2 * D + h * HD : 2 * D + (h + 1) * HD] for h in hs]
        )
        wqkvT = np.ascontiguousarray(np.concatenate(rows, axis=0).T)  # (1024, 768)
        codeT4 = np.zeros((128, 128), dtype=np.float32)
        for j, h in enumerate(hs):
            hh = j % 2  # position within the hd pair
            ct = fc_code[0, h].T  # (64, 16)
            codeT4[64 * hh : 64 * hh + 64, 32 * j : 32 * j + 16] = ct
        woutT = np.ascontiguousarray(
            np.concatenate([w_out[:, h * HD : (h + 1) * HD].T for h in hs], axis=0)
        ) * np.float32(SCALE)  # (256, 1024), post-softmax scale folded in
        in_maps.append(
            {
                "xT": xTs[b],
                "wqkvT": wqkvT.astype(bf),
                "codeT4": codeT4.astype(bf),
                "woutT": woutT.astype(bf),
                "maskT": mask.astype(bf),
                "iden": iden.astype(bf),
                "onehot": onehot.astype(bf),
            }
        )
    return in_maps


def gather(results):
    out = np.zeros((B, T, D), dtype=np.float32)
    for core in range(N_CORES):
        out[core // HC] += np.asarray(results[core]["partial"], dtype=np.float32)
    return out


_NC_CACHE = {}


def kernel(x, w_qkv, w_out, fc_code):
    from concourse.bass_utils import run_bass_kernel_spmd

    if 1 not in _NC_CACHE:
        _NC_CACHE[1] = build(1)
    nc = _NC_CACHE[1]
    in_maps = make_in_maps(x, w_qkv, w_out, fc_code)
    res = run_bass_kernel_spmd(nc, in_maps, list(range(N_CORES)))
    return gather(res.results)
